# revision 20
# baseline (speedup 1.0000x reference)
"""AlphaMixerAttentionHeads TRN2 kernel (pipelined, bf16 matmul operands).

Algebraic structure (verified against the reference, inherited from the
baseline kernel):
 - alpha is i-independent (init ones, update preserves it), so it collapses
   to a per-(b,h) vector u over o; the output is constant across sequence
   positions and equals the m_3 = sum_o H3[:,o] u_3[o] channel vector.
 - W rows are L1-normalized so rec row sums equal H row sums; all per-token
   scales cancel through the NNMF recurrence, which runs on raw clipped xe:
   H_{k+1} = H_k * ((xe / (H_k @ W)) @ W^T), with H_1 = (xe * rec1r) @ W^T
   (rec1r folded into W^T's rows).
 - u_0 = 1/rowsum(H_3); hri = (rec3 * xe) / (sx * s2) feeds the
   per-iteration g = vblk^T hri matmuls; the xe clip keeps only the max
   with 0 (Relu with bias eb-MIN_POS), a ~1e-6 absolute shift that is far
   below the bf16 rounding floor.

Sharding: 8 cores; core c handles batch c//4 and heads 3*(c%4)..3*(c%4)+2
(192 embedding channels). Host sums 4 partial output projections per batch,
adds out_b, broadcasts over the sequence axis.

On-core layout is channel-major [feature, token] in one [128, 1536] tile
set: cols 0..1023 heads A,B (partitions 0-63 = A, 64-127 = B); cols
1024..1535 head C split-token (partitions 0-63 = tokens 0-511, 64-127 =
tokens 512-1023), produced directly by tile_position matmuls in the embed.
All work is chunked in 512-column pieces and software-pipelined across
PE / DVE / ACT / GpSimd. All matmul operands are bf16 (weights and moving
data; ~2.5x cheaper LDWEIGHTS+stream than f32r); PSUM accumulation and the
reciprocal/accumulate chain stay fp32, as does the output projection.
"""

import sys

sys.path.insert(0, "/opt/trn_rl_repo")

import numpy as np

B, S, FIN, E, H = 2, 1024, 768, 768, 12
DH = 64
HPC = 3          # heads per core
EPC = HPC * DH   # embed channels per core (192)
NCORES = 8
MIN_POS = 1e-6
NT = 1536        # merged token columns: 1024 pair + 512 C-split
KT = FIN // 128  # 6 contraction tiles for the embed matmul
CH = 512         # pipeline chunk columns
# wpk packed columns: ebm_p | ebm_c2 | wpair(64) | idstk(64) | ones2(128)
WPK_COLS = 1 + 1 + 64 + 64 + 128

_CACHE = {}


def _build_nc():
    import concourse.bacc as bacc
    import concourse.mybir as mybir
    from concourse.tile import TileContext

    f32 = mybir.dt.float32
    f32r = mybir.dt.float32r
    bf16 = mybir.dt.bfloat16
    Alu = mybir.AluOpType
    Act = mybir.ActivationFunctionType
    AX = mybir.AxisListType

    nc = bacc.Bacc()

    fp8 = mybir.dt.float8e4
    d_xT = nc.declare_dram_parameter("xT", [128, KT, S], fp8, isOutput=False)
    d_ewT = nc.declare_dram_parameter("ewT", [128, KT, EPC], fp8, isOutput=False)
    d_wpk = nc.declare_dram_parameter("wpk", [128, WPK_COLS], f32, isOutput=False)
    d_owa = nc.declare_dram_parameter("owa", [128, FIN], f32, isOutput=False)
    d_owc = nc.declare_dram_parameter("owc", [64, FIN], f32, isOutput=False)
    d_y = nc.declare_dram_parameter("y", [1, FIN], f32, isOutput=True)

    CHUNKS = ((0, 512), (512, 1024), (1024, 1536))

    with TileContext(nc) as tc:
        with (
            tc.tile_pool(name="const", bufs=1) as const,
            tc.tile_pool(name="xch", bufs=3) as xch,
            tc.tile_pool(name="work", bufs=1) as work,
            tc.tile_pool(name="hbuf", bufs=2) as hbuf,
            tc.tile_pool(name="tbuf", bufs=2) as tbuf,
            tc.tile_pool(name="pmm", bufs=3, space="PSUM") as pmm,
            tc.tile_pool(name="pt", bufs=2, space="PSUM") as pt,
        ):
            # ---- DMA triggers: xT on the sync queue (3), weights on the
            # scalar queue (2 now, owT later) so trigger issue is parallel.
            xts = []
            for i in range(3):
                xt = xch.tile([128, 2, S], fp8, tag="xch")
                nc.sync.dma_start(out=xt, in_=d_xT[:, 2 * i:2 * i + 2, :])
                xts.append(xt)
            ewT_sb = const.tile([128, KT, EPC], fp8)
            nc.scalar.dma_start(out=ewT_sb, in_=d_ewT[:, :, :])
            wpk = const.tile([128, WPK_COLS], f32)
            nc.scalar.dma_start(out=wpk, in_=d_wpk[:, :])

            ebm_p = wpk[:, 0:1]
            ebm_c2 = wpk[:, 1:2]
            wpair = wpk[:, 2:66]
            idstk = wpk[:, 66:130]
            ones2f = wpk[:, 130:258]

            # ---- W prep: fp32 masters on DVE, bf16 matmul copies via ACT
            W2f = const.tile([128, 128], f32)
            W2Tf = const.tile([128, 128], f32)
            Wstk2 = const.tile([128, 128], f32)
            W2b = const.tile([128, 128], bf16)
            W2Tb = const.tile([128, 128], bf16)
            W2Tpb = const.tile([128, 128], bf16)
            ones2b = const.tile([128, 128], bf16)
            vblk = const.tile([128, 128], bf16)
            vblkC = const.tile([128, 128], bf16)
            nc.scalar.activation(out=W2f, in_=wpk[:, 0:128], func=Act.Copy, scale=0.0)
            nc.scalar.activation(out=W2Tf, in_=wpk[:, 0:128], func=Act.Copy, scale=0.0)
            nc.scalar.activation(out=vblk, in_=wpk[:, 0:128], func=Act.Copy, scale=0.0)
            nc.scalar.activation(out=vblkC, in_=wpk[:, 0:128], func=Act.Copy, scale=0.0)
            nc.scalar.activation(out=ones2b, in_=ones2f, func=Act.Copy)

            wsum = work.tile([128, 1], f32)
            nc.vector.reduce_sum(out=wsum, in_=wpair, axis=AX.X)
            wrec = work.tile([128, 1], f32)
            nc.vector.reciprocal_approx_fast(out=wrec, in_=wsum)
            nc.vector.tensor_scalar(
                out=W2f[0:64, 0:64], in0=wpair[0:64, :], scalar1=wrec[0:64, :],
                scalar2=None, op0=Alu.mult,
            )
            nc.vector.tensor_scalar(
                out=W2f[64:128, 64:128], in0=wpair[64:128, :],
                scalar1=wrec[64:128, :], scalar2=None, op0=Alu.mult,
            )
            nc.vector.tensor_scalar(
                out=Wstk2[:, 0:64], in0=wpair, scalar1=wrec,
                scalar2=None, op0=Alu.mult,
            )
            nc.vector.tensor_scalar(
                out=Wstk2[:, 64:128], in0=wpair, scalar1=wrec,
                scalar2=None, op0=Alu.mult,
            )
            nc.scalar.activation(out=W2b, in_=W2f, func=Act.Copy)

            # ---- W2T: one PE transpose for the top block; the bottom
            # diag block is identical, replicated via a gpsimd SBUF copy.
            ps_t = pt.tile([128, 64], f32, tag="tr", bufs=1)
            nc.tensor.transpose(
                out=ps_t[0:64, :], in_=W2f[0:64, 0:64], identity=idstk[0:64, :]
            )
            nc.scalar.activation(out=W2Tf[0:64, 0:64], in_=ps_t[0:64, :], func=Act.Copy)
            nc.tensor.matmul(
                out=ps_t[64:128, :], lhsT=idstk[0:64, :],
                rhs=W2Tf[0:64, 0:64], skip_group_check=True,
            )
            nc.scalar.activation(
                out=W2Tf[64:128, 64:128], in_=ps_t[64:128, :], func=Act.Copy
            )
            nc.scalar.activation(out=W2Tb, in_=W2Tf, func=Act.Copy)

            # rec1r = 64/rowsum(W2T); W2Tp = W2T * rec1r (iter-1 fold)
            rec1s = work.tile([128, 1], f32)
            nc.vector.reduce_sum(out=rec1s, in_=W2Tf, axis=AX.X)
            rec1sc = work.tile([128, 1], f32)
            nc.vector.tensor_scalar(
                out=rec1sc, in0=rec1s, scalar1=1.0 / 64.0, scalar2=None,
                op0=Alu.mult,
            )
            rec1r = work.tile([128, 1], f32)
            nc.vector.reciprocal_approx_fast(out=rec1r, in_=rec1sc)
            nc.vector.tensor_scalar(
                out=W2Tpb, in0=W2Tf, scalar1=rec1r, scalar2=None,
                op0=Alu.mult,
            )

            # ---- embed matmuls: ep = pair heads [128, 1024];
            # psC = head C split-token [128, 512] built in place via
            # partition-offset (tile_position) matmuls.
            ep = pmm.tile([128, 1024], f32, tag="ep", bufs=1)
            psC = pmm.tile([128, CH], f32, tag="pc", bufs=1)
            DR = mybir.MatmulPerfMode.DoubleRow
            for g in range(3):
                xt = xts[g]
                lhsP2 = ewT_sb[:, 2 * g:2 * g + 2, 0:128]
                st = dict(start=(g == 0), stop=(g == 2), perf_mode=DR)
                nc.tensor.matmul(
                    out=ep[:, 0:512], lhsT=lhsP2, rhs=xt[:, :, 0:512], **st
                )
                nc.tensor.matmul(
                    out=ep[:, 512:1024], lhsT=lhsP2, rhs=xt[:, :, 512:1024], **st
                )
            for k in range(KT):
                xt = xts[k // 2][:, k % 2, :]
                lhsC = ewT_sb[:, k, 128:192]
                st2 = dict(start=(k == 0), stop=(k == KT - 1))
                nc.tensor.matmul(
                    out=psC[0:64, :], lhsT=lhsC, rhs=xt[:, 0:512],
                    skip_group_check=True, **st2,
                )
                nc.tensor.matmul(
                    out=psC[64:128, :], lhsT=lhsC, rhs=xt[:, 512:1024],
                    skip_group_check=True, **st2,
                )

            # ---- xe = relu(embed + eb - MIN_POS) on ACT (bias pre-folded)
            xe = work.tile([128, NT], bf16)
            nc.scalar.activation(
                out=xe[:, 0:512], in_=ep[:, 0:512], func=Act.Relu,
                bias=ebm_p, scale=1.0 / 64.0,
            )
            nc.scalar.activation(
                out=xe[:, 512:1024], in_=ep[:, 512:1024], func=Act.Relu,
                bias=ebm_p, scale=1.0 / 64.0,
            )
            nc.scalar.activation(
                out=xe[:, 1024:1536], in_=psC, func=Act.Relu,
                bias=ebm_c2, scale=1.0 / 64.0,
            )

            # ---- NNMF iter 1: H1 = xe @ (W^T * rec1r), plus sx row sums
            z1s = []
            for lo, hi in CHUNKS:
                z = pmm.tile([128, CH], f32, tag="mm")
                nc.tensor.matmul(out=z, lhsT=W2Tpb, rhs=xe[:, lo:hi])
                z1s.append(z)
            sxs_ps = []
            for lo, hi in CHUNKS:
                sx = pmm.tile([128, CH], f32, tag="mm")
                nc.tensor.matmul(out=sx, lhsT=ones2b, rhs=xe[:, lo:hi])
                sxs_ps.append(sx)
            Hc = hbuf.tile([128, NT], bf16, tag="h")
            for ci, (lo, hi) in enumerate(CHUNKS):
                nc.scalar.activation(out=Hc[:, lo:hi], in_=z1s[ci], func=Act.Copy)
            sxs = work.tile([128, NT], f32)
            for ci, (lo, hi) in enumerate(CHUNKS):
                nc.scalar.activation(out=sxs[:, lo:hi], in_=sxs_ps[ci], func=Act.Copy)

            # ---- NNMF iters 2-3 (chunk-pipelined); iter 3 also builds hri
            owa = const.tile([128, FIN], f32r)
            owc = const.tile([64, FIN], f32r)
            rec3s = work.tile([128, NT], bf16)
            sxs2 = work.tile([128, NT], f32)
            hri_raw = work.tile([128, NT], bf16)
            hri = work.tile([128, NT], bf16)
            for it in range(1, 3):
                last = it == 2
                recs = []
                for lo, hi in CHUNKS:
                    rec = pmm.tile([128, CH], f32, tag="mm")
                    nc.tensor.matmul(out=rec, lhsT=W2b, rhs=Hc[:, lo:hi])
                    recs.append(rec)
                q = work.tile([128, NT], bf16, tag="q")
                rr = work.tile([128, NT], f32, tag="rr")
                for ci, (lo, hi) in enumerate(CHUNKS):
                    nc.vector.reciprocal_approx_fast(out=rr[:, lo:hi], in_=recs[ci])
                    eng = nc.gpsimd if ci == 2 else nc.vector
                    eng.tensor_tensor(
                        out=q[:, lo:hi], in0=xe[:, lo:hi], in1=rr[:, lo:hi],
                        op=Alu.mult,
                    )
                if last:
                    for ci, (lo, hi) in enumerate(CHUNKS):
                        nc.scalar.activation(
                            out=rec3s[:, lo:hi], in_=recs[ci], func=Act.Copy
                        )
                    nc.scalar.dma_start(out=owa, in_=d_owa[:, :].bitcast(f32r))
                    nc.scalar.dma_start(out=owc, in_=d_owc[:, :].bitcast(f32r))
                zs = []
                for lo, hi in CHUNKS:
                    z = pmm.tile([128, CH], f32, tag="mm")
                    nc.tensor.matmul(out=z, lhsT=W2Tb, rhs=q[:, lo:hi])
                    zs.append(z)
                Hn = hbuf.tile([128, NT], bf16, tag="h")
                for ci, (lo, hi) in enumerate(CHUNKS):
                    nc.vector.tensor_tensor(
                        out=Hn[:, lo:hi], in0=Hc[:, lo:hi],
                        in1=zs[ci], op=Alu.mult,
                    )
                if not last:
                    # s2 row sums of H2 (side path for hri)
                    s2_ps = []
                    for lo, hi in CHUNKS:
                        s2 = pmm.tile([128, CH], f32, tag="mm")
                        nc.tensor.matmul(out=s2, lhsT=ones2b, rhs=Hn[:, lo:hi])
                        s2_ps.append(s2)
                    for ci, (lo, hi) in enumerate(CHUNKS):
                        nc.vector.tensor_tensor(
                            out=sxs2[:, lo:hi], in0=sxs[:, lo:hi],
                            in1=s2_ps[ci], op=Alu.mult,
                        )
                else:
                    # hri = (rec3 * xe) / (sx * s2)
                    for lo, hi in CHUNKS:
                        nc.gpsimd.tensor_tensor(
                            out=hri_raw[:, lo:hi], in0=rec3s[:, lo:hi],
                            in1=xe[:, lo:hi], op=Alu.mult,
                        )
                    rho2 = work.tile([128, NT], f32)
                    for lo, hi in CHUNKS:
                        nc.vector.reciprocal_approx_fast(
                            out=rho2[:, lo:hi], in_=sxs2[:, lo:hi]
                        )
                    for ci, (lo, hi) in enumerate(CHUNKS):
                        eng = nc.gpsimd if ci == 2 else nc.vector
                        eng.tensor_tensor(
                            out=hri[:, lo:hi], in0=hri_raw[:, lo:hi],
                            in1=rho2[:, lo:hi], op=Alu.mult,
                        )
                Hc = Hn

            # ---- s3 row sums of H3, u0 = 1/s3
            s3_ps = []
            for lo, hi in CHUNKS:
                s3 = pmm.tile([128, CH], f32, tag="mm")
                nc.tensor.matmul(out=s3, lhsT=ones2b, rhs=Hc[:, lo:hi])
                s3_ps.append(s3)
            u0 = work.tile([128, NT], f32)
            for ci, (lo, hi) in enumerate(CHUNKS):
                nc.vector.reciprocal_approx_fast(out=u0[:, lo:hi], in_=s3_ps[ci])

            # ---- alpha fixed point: 4 accumulation passes, 3 v/g rounds
            vv = pt.tile([128, 4], f32, tag="v", bufs=1)
            c_p = work.tile([128, 1], f32)
            c_cc = work.tile([128, 1], f32)
            t_prev = None
            g_ps = None
            for it in range(4):
                lastit = it == 3
                t = tbuf.tile([128, NT], f32, tag="t")
                in0 = Hc if it == 0 else t_prev
                macc = []
                for ci, (lo, hi) in enumerate(CHUNKS):
                    in1 = u0[:, lo:hi] if it == 0 else g_ps[ci]
                    m = work.tile([128, 1], f32, tag=f"m{it}{ci}")
                    nc.vector.scalar_tensor_tensor(
                        out=t[:, lo:hi], in0=in0[:, lo:hi], scalar=1.0,
                        in1=in1, op0=Alu.mult, op1=Alu.mult, accum_out=m,
                    )
                    macc.append(m)
                t_prev = t
                m_cc = macc[2]
                if lastit:
                    nc.vector.tensor_tensor(
                        out=c_p, in0=macc[0], in1=macc[1], op=Alu.add
                    )
                    nc.vector.tensor_copy(out=c_cc, in_=m_cc)
                    break
                vps = vv[:, 0:1]
                nc.tensor.matmul(
                    out=vps, lhsT=W2f, rhs=macc[0], start=True, stop=False,
                    skip_group_check=True,
                )
                nc.tensor.matmul(
                    out=vps, lhsT=W2f, rhs=macc[1], start=False, stop=True,
                    skip_group_check=True,
                )
                vcs = vv[:, 1:2]
                nc.tensor.matmul(out=vcs, lhsT=Wstk2, rhs=m_cc, skip_group_check=True)
                v_p = work.tile([128, 1], f32, tag="v_p")
                v_c = work.tile([128, 1], f32, tag="v_c")
                nc.vector.reciprocal_approx_fast(out=v_p, in_=vps)
                nc.vector.reciprocal_approx_fast(out=v_c, in_=vcs)
                nc.scalar.activation(
                    out=vblk, in_=ones2f, func=Act.Copy, scale=v_p
                )
                nc.scalar.activation(
                    out=vblkC, in_=ones2f, func=Act.Copy, scale=v_c
                )
                g_ps = []
                for ci, (lo, hi) in enumerate(CHUNKS):
                    g = pmm.tile([128, CH], f32, tag="mm")
                    nc.tensor.matmul(
                        out=g, lhsT=(vblkC if ci == 2 else vblk),
                        rhs=hri[:, lo:hi],
                    )
                    g_ps.append(g)

            # fold the C accumulator's split halves: c_c[f] = acc[f]+acc[64+f]
            fc = vv[0:64, 2:3]
            nc.tensor.matmul(out=fc, lhsT=idstk, rhs=c_cc, skip_group_check=True)
            c_c = work.tile([64, 1], f32r)
            nc.scalar.activation(out=c_c, in_=fc, func=Act.Copy)

            # ---- output projection partial: y_row = c^T @ owT  [1, FIN]
            c_pr = work.tile([128, 1], f32r)
            nc.vector.tensor_copy(out=c_pr, in_=c_p)
            py0 = pmm.tile([1, 512], f32, tag="mm")
            py1 = pmm.tile([1, 256], f32, tag="mm")
            for py, (lo, hi) in ((py0, (0, 512)), (py1, (512, FIN))):
                nc.tensor.matmul(
                    out=py, lhsT=c_pr, rhs=owa[:, lo:hi], start=True, stop=False
                )
                nc.tensor.matmul(
                    out=py, lhsT=c_c, rhs=owc[:, lo:hi], start=False, stop=True
                )
            y_sb = work.tile([1, FIN], f32)
            nc.scalar.activation(out=y_sb[:, 0:512], in_=py0, func=Act.Copy)
            nc.scalar.activation(out=y_sb[:, 512:FIN], in_=py1, func=Act.Copy)
            nc.sync.dma_start(out=d_y[:, :], in_=y_sb)

    nc.finalize()
    return nc


def _make_in_maps(x, embed_w, embed_b, nnmf_w, out_w):
    def to_fp8(a):
        import ml_dtypes
        return np.ascontiguousarray(a).astype(ml_dtypes.float8_e4m3fn)

    idstk = np.zeros((128, 64), np.float32)
    for k in range(128):
        idstk[k, k % 64] = 1.0
    ones2 = np.zeros((128, 128), np.float32)
    ones2[0:64, 0:64] = 1.0
    ones2[64:128, 64:128] = 1.0

    in_maps = []
    for c in range(NCORES):
        b = c // 4
        hg = c % 4
        esl = slice(EPC * hg, EPC * (hg + 1))
        # xT packed [128, KT, S]: (p, k, t) = x[b, t, 128k+p]
        xT = np.ascontiguousarray(
            x[b].T.reshape(KT, 128, S).transpose(1, 0, 2)
        )
        ewT = np.ascontiguousarray(
            embed_w[esl, :].T.reshape(KT, 128, EPC).transpose(1, 0, 2)
        )
        ebm = embed_b[esl] - MIN_POS
        wpk = np.zeros((128, WPK_COLS), np.float32)
        wpk[:, 0] = ebm[0:128]
        wpk[0:64, 1] = ebm[128:192]
        wpk[64:128, 1] = ebm[128:192]
        wpk[0:64, 2:66] = nnmf_w
        wpk[64:128, 2:66] = nnmf_w
        wpk[:, 66:130] = idstk
        wpk[:, 130:258] = ones2
        owT = out_w[:, esl].T  # [192, FIN]
        in_maps.append({
            "xT": to_fp8(xT),
            "ewT": to_fp8(ewT * 64.0),
            "wpk": wpk,
            "owa": np.ascontiguousarray(owT[0:128, :]),
            "owc": np.ascontiguousarray(owT[128:192, :]),
        })
    return in_maps


def _ensure_ntff_hook():
    """The agent image's antenv lacks axon_hooks; synthesize it so
    run_bass_kernel_spmd(trace=True) can reach the ctypes NTFF hook."""
    import sys as _sys
    import types

    if "antenv.axon_hooks" in _sys.modules:
        return
    mod = types.ModuleType("antenv.axon_hooks")
    holder = [None]
    mod.set_axon_ntff_profile_hook = lambda h: holder.__setitem__(0, h)
    mod.get_axon_ntff_profile_hook = lambda: holder[0]
    _sys.modules["antenv.axon_hooks"] = mod
    try:
        import antenv

        antenv.axon_hooks = mod
    except ImportError:
        pass
    from trn_agent_boot.trn_boot import _ntff_profile_via_ctypes

    mod.set_axon_ntff_profile_hook(
        _ntff_profile_via_ctypes("/opt/axon/libaxon_pjrt.so")
    )


def _run(inputs, trace=False):
    from concourse import bass_utils

    if trace:
        _ensure_ntff_hook()
    if "nc" not in _CACHE:
        _CACHE["nc"] = _build_nc()
    nc = _CACHE["nc"]
    in_maps = _make_in_maps(
        inputs["x"].astype(np.float32),
        inputs["embed_w"].astype(np.float32),
        inputs["embed_b"].astype(np.float32),
        inputs["nnmf_w"].astype(np.float32),
        inputs["out_w"].astype(np.float32),
    )
    res = bass_utils.run_bass_kernel_spmd(
        nc, in_maps, core_ids=list(range(NCORES)), trace=trace
    )
    out_b = inputs["out_b"].astype(np.float32)
    y = np.zeros((B, S, FIN), np.float32)
    for bi in range(B):
        acc = np.zeros((FIN,), np.float64)
        for c in range(4 * bi, 4 * bi + 4):
            arr = np.asarray(res.results[c]["y"])  # [1, FIN]
            acc += arr.reshape(FIN)
        y[bi, :, :] = (acc + out_b).astype(np.float32)[None, :]
    return y, res


def kernel(**inputs):
    y, _ = _run(inputs, trace=False)
    return y


# revision 22
# speedup vs baseline: 1.0072x; 1.0072x over previous
"""AlphaMixerAttentionHeads TRN2 kernel (pipelined, bf16 matmul operands).

Algebraic structure (verified against the reference, inherited from the
baseline kernel):
 - alpha is i-independent (init ones, update preserves it), so it collapses
   to a per-(b,h) vector u over o; the output is constant across sequence
   positions and equals the m_3 = sum_o H3[:,o] u_3[o] channel vector.
 - W rows are L1-normalized so rec row sums equal H row sums; all per-token
   scales cancel through the NNMF recurrence, which runs on raw clipped xe:
   H_{k+1} = H_k * ((xe / (H_k @ W)) @ W^T), with H_1 = (xe * rec1r) @ W^T
   (rec1r folded into W^T's rows).
 - u_0 = 1/rowsum(H_3); hri = (rec3 * xe) / (sx * s2) feeds the
   per-iteration g = vblk^T hri matmuls; the xe clip keeps only the max
   with 0 (Relu with bias eb-MIN_POS), a ~1e-6 absolute shift that is far
   below the bf16 rounding floor.

Sharding: 8 cores; core c handles batch c//4 and heads 3*(c%4)..3*(c%4)+2
(192 embedding channels). Host sums 4 partial output projections per batch,
adds out_b, broadcasts over the sequence axis.

On-core layout is channel-major [feature, token] in one [128, 1536] tile
set: cols 0..1023 heads A,B (partitions 0-63 = A, 64-127 = B); cols
1024..1535 head C split-token (partitions 0-63 = tokens 0-511, 64-127 =
tokens 512-1023), produced directly by tile_position matmuls in the embed.
All work is chunked in 512-column pieces and software-pipelined across
PE / DVE / ACT / GpSimd. All matmul operands are bf16 (weights and moving
data; ~2.5x cheaper LDWEIGHTS+stream than f32r); PSUM accumulation and the
reciprocal/accumulate chain stay fp32, as does the output projection.
"""

import sys

sys.path.insert(0, "/opt/trn_rl_repo")

import numpy as np

B, S, FIN, E, H = 2, 1024, 768, 768, 12
DH = 64
HPC = 3          # heads per core
EPC = HPC * DH   # embed channels per core (192)
NCORES = 8
MIN_POS = 1e-6
NT = 1536        # merged token columns: 1024 pair + 512 C-split
KT = FIN // 128  # 6 contraction tiles for the embed matmul
CH = 512         # pipeline chunk columns
# wpk packed columns: ebm_p | ebm_c2 | wpair(64) | idstk(64) | ones2(128)
WPK_COLS = 1 + 1 + 64 + 64 + 128

_CACHE = {}


def _build_nc():
    import concourse.bacc as bacc
    import concourse.mybir as mybir
    from concourse.tile import TileContext

    f32 = mybir.dt.float32
    f32r = mybir.dt.float32r
    bf16 = mybir.dt.bfloat16
    Alu = mybir.AluOpType
    Act = mybir.ActivationFunctionType
    AX = mybir.AxisListType

    nc = bacc.Bacc()

    fp8 = mybir.dt.float8e4
    d_xT = nc.declare_dram_parameter("xT", [128, KT, S], fp8, isOutput=False)
    d_ewT = nc.declare_dram_parameter("ewT", [128, KT, EPC], fp8, isOutput=False)
    d_wpk = nc.declare_dram_parameter("wpk", [128, WPK_COLS], f32, isOutput=False)
    d_owa = nc.declare_dram_parameter("owa", [128, FIN], f32, isOutput=False)
    d_owc = nc.declare_dram_parameter("owc", [64, FIN], f32, isOutput=False)
    d_y = nc.declare_dram_parameter("y", [1, FIN], f32, isOutput=True)

    CHUNKS = ((0, 512), (512, 1024), (1024, 1536))

    with TileContext(nc) as tc:
        with (
            tc.tile_pool(name="const", bufs=1) as const,
            tc.tile_pool(name="xch", bufs=3) as xch,
            tc.tile_pool(name="work", bufs=1) as work,
            tc.tile_pool(name="hbuf", bufs=2) as hbuf,
            tc.tile_pool(name="tbuf", bufs=2) as tbuf,
            tc.tile_pool(name="pmm", bufs=3, space="PSUM") as pmm,
            tc.tile_pool(name="pt", bufs=2, space="PSUM") as pt,
        ):
            # ---- DMA triggers: xT on the sync queue (3), weights on the
            # scalar queue (2 now, owT later) so trigger issue is parallel.
            xts = []
            for i in range(3):
                xt = xch.tile([128, 2, S], fp8, tag="xch")
                nc.sync.dma_start(out=xt, in_=d_xT[:, 2 * i:2 * i + 2, :])
                xts.append(xt)
            ewT_sb = const.tile([128, KT, EPC], fp8)
            nc.scalar.dma_start(out=ewT_sb, in_=d_ewT[:, :, :])
            wpk = const.tile([128, WPK_COLS], f32)
            nc.scalar.dma_start(out=wpk, in_=d_wpk[:, :])

            ebm_p = wpk[:, 0:1]
            ebm_c2 = wpk[:, 1:2]
            wpair = wpk[:, 2:66]
            idstk = wpk[:, 66:130]
            ones2f = wpk[:, 130:258]

            # ---- W prep: fp32 masters on DVE, bf16 matmul copies via ACT
            W2f = const.tile([128, 128], f32)
            W2Tf = const.tile([128, 128], f32)
            Wstk2 = const.tile([128, 128], f32)
            W2b = const.tile([128, 128], bf16)
            W2Tb = const.tile([128, 128], bf16)
            W2Tpb = const.tile([128, 128], bf16)
            ones2b = const.tile([128, 128], bf16)
            vblk = const.tile([128, 128], bf16)
            vblkC = const.tile([128, 128], bf16)
            nc.scalar.activation(out=W2f, in_=wpk[:, 0:128], func=Act.Copy, scale=0.0)
            nc.scalar.activation(out=W2Tf, in_=wpk[:, 0:128], func=Act.Copy, scale=0.0)
            nc.scalar.activation(out=vblk, in_=wpk[:, 0:128], func=Act.Copy, scale=0.0)
            nc.scalar.activation(out=vblkC, in_=wpk[:, 0:128], func=Act.Copy, scale=0.0)
            nc.scalar.activation(out=ones2b, in_=ones2f, func=Act.Copy)

            wsum = work.tile([128, 1], f32)
            nc.vector.reduce_sum(out=wsum, in_=wpair, axis=AX.X)
            wrec = work.tile([128, 1], f32)
            nc.vector.reciprocal_approx_fast(out=wrec, in_=wsum)
            nc.vector.tensor_scalar(
                out=W2f[0:64, 0:64], in0=wpair[0:64, :], scalar1=wrec[0:64, :],
                scalar2=None, op0=Alu.mult,
            )
            nc.vector.tensor_scalar(
                out=W2f[64:128, 64:128], in0=wpair[64:128, :],
                scalar1=wrec[64:128, :], scalar2=None, op0=Alu.mult,
            )
            nc.vector.tensor_scalar(
                out=Wstk2[:, 0:64], in0=wpair, scalar1=wrec,
                scalar2=None, op0=Alu.mult,
            )
            nc.vector.tensor_scalar(
                out=Wstk2[:, 64:128], in0=wpair, scalar1=wrec,
                scalar2=None, op0=Alu.mult,
            )
            nc.scalar.activation(out=W2b, in_=W2f, func=Act.Copy)

            # ---- W2T: one PE transpose for the top block; the bottom
            # diag block is identical, replicated via a gpsimd SBUF copy.
            ps_t = pt.tile([128, 64], f32, tag="tr", bufs=1)
            nc.tensor.transpose(
                out=ps_t[0:64, :], in_=W2f[0:64, 0:64], identity=idstk[0:64, :]
            )
            nc.scalar.activation(out=W2Tf[0:64, 0:64], in_=ps_t[0:64, :], func=Act.Copy)
            nc.tensor.matmul(
                out=ps_t[64:128, :], lhsT=idstk[0:64, :],
                rhs=W2Tf[0:64, 0:64], skip_group_check=True,
            )
            nc.scalar.activation(
                out=W2Tf[64:128, 64:128], in_=ps_t[64:128, :], func=Act.Copy
            )
            nc.scalar.activation(out=W2Tb, in_=W2Tf, func=Act.Copy)

            # rec1r = 64/rowsum(W2T); W2Tp = W2T * rec1r (iter-1 fold)
            rec1s = work.tile([128, 1], f32)
            nc.vector.reduce_sum(out=rec1s, in_=W2Tf, axis=AX.X)
            rec1sc = work.tile([128, 1], f32)
            nc.vector.tensor_scalar(
                out=rec1sc, in0=rec1s, scalar1=1.0 / 64.0, scalar2=None,
                op0=Alu.mult,
            )
            rec1r = work.tile([128, 1], f32)
            nc.vector.reciprocal_approx_fast(out=rec1r, in_=rec1sc)
            nc.vector.tensor_scalar(
                out=W2Tpb, in0=W2Tf, scalar1=rec1r, scalar2=None,
                op0=Alu.mult,
            )

            # ---- embed matmuls: ep = pair heads [128, 1024];
            # psC = head C split-token [128, 512] built in place via
            # partition-offset (tile_position) matmuls.
            ep = pmm.tile([128, 1024], f32, tag="ep", bufs=1)
            psC = pmm.tile([128, CH], f32, tag="pc", bufs=1)
            DR = mybir.MatmulPerfMode.DoubleRow
            for g in range(3):
                xt = xts[g]
                lhsP2 = ewT_sb[:, 2 * g:2 * g + 2, 0:128]
                st = dict(start=(g == 0), stop=(g == 2), perf_mode=DR)
                nc.tensor.matmul(
                    out=ep[:, 0:512], lhsT=lhsP2, rhs=xt[:, :, 0:512], **st
                )
                nc.tensor.matmul(
                    out=ep[:, 512:1024], lhsT=lhsP2, rhs=xt[:, :, 512:1024], **st
                )
            for k in range(KT):
                xt = xts[k // 2][:, k % 2, :]
                lhsC = ewT_sb[:, k, 128:192]
                st2 = dict(start=(k == 0), stop=(k == KT - 1))
                nc.tensor.matmul(
                    out=psC[0:64, :], lhsT=lhsC, rhs=xt[:, 0:512],
                    skip_group_check=True, **st2,
                )
                nc.tensor.matmul(
                    out=psC[64:128, :], lhsT=lhsC, rhs=xt[:, 512:1024],
                    skip_group_check=True, **st2,
                )

            # ---- xe = relu(embed + eb - MIN_POS) on ACT (bias pre-folded)
            xe = work.tile([128, NT], bf16)
            nc.scalar.activation(
                out=xe[:, 0:512], in_=ep[:, 0:512], func=Act.Relu,
                bias=ebm_p, scale=1.0 / 64.0,
            )
            nc.scalar.activation(
                out=xe[:, 512:1024], in_=ep[:, 512:1024], func=Act.Relu,
                bias=ebm_p, scale=1.0 / 64.0,
            )
            nc.scalar.activation(
                out=xe[:, 1024:1536], in_=psC, func=Act.Relu,
                bias=ebm_c2, scale=1.0 / 64.0,
            )

            # ---- NNMF iter 1: H1 = xe @ (W^T * rec1r), plus sx row sums
            z1s = []
            for lo, hi in CHUNKS:
                z = pmm.tile([128, CH], f32, tag="mm")
                nc.tensor.matmul(out=z, lhsT=W2Tpb, rhs=xe[:, lo:hi])
                z1s.append(z)
            sxs_ps = []
            for lo, hi in CHUNKS:
                sx = pmm.tile([128, CH], f32, tag="mm")
                nc.tensor.matmul(out=sx, lhsT=ones2b, rhs=xe[:, lo:hi])
                sxs_ps.append(sx)
            Hc = hbuf.tile([128, NT], bf16, tag="h")
            for ci, (lo, hi) in enumerate(CHUNKS):
                nc.scalar.activation(out=Hc[:, lo:hi], in_=z1s[ci], func=Act.Copy)
            sxs = work.tile([128, NT], f32)
            for ci, (lo, hi) in enumerate(CHUNKS):
                nc.scalar.activation(out=sxs[:, lo:hi], in_=sxs_ps[ci], func=Act.Copy)

            # ---- NNMF iters 2-3 (chunk-pipelined); iter 3 also builds hri
            owa = const.tile([128, FIN], f32r)
            owc = const.tile([64, FIN], f32r)
            rec3s = work.tile([128, NT], bf16)
            sxs2 = work.tile([128, NT], f32)
            hri_raw = work.tile([128, NT], bf16)
            hri = work.tile([128, NT], bf16)
            for it in range(1, 3):
                last = it == 2
                recs = []
                for lo, hi in CHUNKS:
                    rec = pmm.tile([128, CH], f32, tag="mm")
                    nc.tensor.matmul(out=rec, lhsT=W2b, rhs=Hc[:, lo:hi])
                    recs.append(rec)
                q = work.tile([128, NT], bf16, tag="q")
                rr = work.tile([128, NT], f32, tag="rr")
                for ci, (lo, hi) in enumerate(CHUNKS):
                    nc.vector.reciprocal_approx_fast(out=rr[:, lo:hi], in_=recs[ci])
                    on_pool = ci == 2 or (ci == 1 and not last)
                    eng = nc.gpsimd if on_pool else nc.vector
                    eng.tensor_tensor(
                        out=q[:, lo:hi], in0=xe[:, lo:hi], in1=rr[:, lo:hi],
                        op=Alu.mult,
                    )
                if last:
                    for ci, (lo, hi) in enumerate(CHUNKS):
                        nc.scalar.activation(
                            out=rec3s[:, lo:hi], in_=recs[ci], func=Act.Copy
                        )
                    nc.scalar.dma_start(out=owa, in_=d_owa[:, :].bitcast(f32r))
                    nc.scalar.dma_start(out=owc, in_=d_owc[:, :].bitcast(f32r))
                zs = []
                for lo, hi in CHUNKS:
                    z = pmm.tile([128, CH], f32, tag="mm")
                    nc.tensor.matmul(out=z, lhsT=W2Tb, rhs=q[:, lo:hi])
                    zs.append(z)
                Hn = hbuf.tile([128, NT], bf16, tag="h")
                for ci, (lo, hi) in enumerate(CHUNKS):
                    nc.vector.tensor_tensor(
                        out=Hn[:, lo:hi], in0=Hc[:, lo:hi],
                        in1=zs[ci], op=Alu.mult,
                    )
                if not last:
                    # s2 row sums of H2 (side path for hri)
                    s2_ps = []
                    for lo, hi in CHUNKS:
                        s2 = pmm.tile([128, CH], f32, tag="mm")
                        nc.tensor.matmul(out=s2, lhsT=ones2b, rhs=Hn[:, lo:hi])
                        s2_ps.append(s2)
                    for ci, (lo, hi) in enumerate(CHUNKS):
                        nc.vector.tensor_tensor(
                            out=sxs2[:, lo:hi], in0=sxs[:, lo:hi],
                            in1=s2_ps[ci], op=Alu.mult,
                        )
                else:
                    # hri = (rec3 * xe) / (sx * s2)
                    for lo, hi in CHUNKS:
                        nc.gpsimd.tensor_tensor(
                            out=hri_raw[:, lo:hi], in0=rec3s[:, lo:hi],
                            in1=xe[:, lo:hi], op=Alu.mult,
                        )
                    rho2 = work.tile([128, NT], f32)
                    for lo, hi in CHUNKS:
                        nc.vector.reciprocal_approx_fast(
                            out=rho2[:, lo:hi], in_=sxs2[:, lo:hi]
                        )
                    for ci, (lo, hi) in enumerate(CHUNKS):
                        eng = nc.gpsimd if ci == 2 else nc.vector
                        eng.tensor_tensor(
                            out=hri[:, lo:hi], in0=hri_raw[:, lo:hi],
                            in1=rho2[:, lo:hi], op=Alu.mult,
                        )
                Hc = Hn

            # ---- s3 row sums of H3, u0 = 1/s3
            s3_ps = []
            for lo, hi in CHUNKS:
                s3 = pmm.tile([128, CH], f32, tag="mm")
                nc.tensor.matmul(out=s3, lhsT=ones2b, rhs=Hc[:, lo:hi])
                s3_ps.append(s3)
            u0 = work.tile([128, NT], f32)
            for ci, (lo, hi) in enumerate(CHUNKS):
                nc.vector.reciprocal_approx_fast(out=u0[:, lo:hi], in_=s3_ps[ci])

            # ---- alpha fixed point: 4 accumulation passes, 3 v/g rounds
            vv = pt.tile([128, 4], f32, tag="v", bufs=1)
            c_p = work.tile([128, 1], f32)
            c_cc = work.tile([128, 1], f32)
            t_prev = None
            g_ps = None
            for it in range(4):
                lastit = it == 3
                t = tbuf.tile([128, NT], f32, tag="t")
                in0 = Hc if it == 0 else t_prev
                macc = []
                for ci, (lo, hi) in enumerate(CHUNKS):
                    in1 = u0[:, lo:hi] if it == 0 else g_ps[ci]
                    m = work.tile([128, 1], f32, tag=f"m{it}{ci}")
                    nc.vector.scalar_tensor_tensor(
                        out=t[:, lo:hi], in0=in0[:, lo:hi], scalar=1.0,
                        in1=in1, op0=Alu.mult, op1=Alu.mult, accum_out=m,
                    )
                    macc.append(m)
                t_prev = t
                m_cc = macc[2]
                if lastit:
                    nc.vector.tensor_tensor(
                        out=c_p, in0=macc[0], in1=macc[1], op=Alu.add
                    )
                    nc.vector.tensor_copy(out=c_cc, in_=m_cc)
                    break
                vps = vv[:, 0:1]
                nc.tensor.matmul(
                    out=vps, lhsT=W2f, rhs=macc[0], start=True, stop=False,
                    skip_group_check=True,
                )
                nc.tensor.matmul(
                    out=vps, lhsT=W2f, rhs=macc[1], start=False, stop=True,
                    skip_group_check=True,
                )
                vcs = vv[:, 1:2]
                nc.tensor.matmul(out=vcs, lhsT=Wstk2, rhs=m_cc, skip_group_check=True)
                v_p = work.tile([128, 1], f32, tag="v_p")
                v_c = work.tile([128, 1], f32, tag="v_c")
                nc.vector.reciprocal_approx_fast(out=v_p, in_=vps)
                nc.vector.reciprocal_approx_fast(out=v_c, in_=vcs)
                nc.scalar.activation(
                    out=vblk, in_=ones2f, func=Act.Copy, scale=v_p
                )
                nc.scalar.activation(
                    out=vblkC, in_=ones2f, func=Act.Copy, scale=v_c
                )
                g_ps = []
                for ci, (lo, hi) in enumerate(CHUNKS):
                    g = pmm.tile([128, CH], f32, tag="mm")
                    nc.tensor.matmul(
                        out=g, lhsT=(vblkC if ci == 2 else vblk),
                        rhs=hri[:, lo:hi],
                    )
                    g_ps.append(g)

            # fold the C accumulator's split halves: c_c[f] = acc[f]+acc[64+f]
            fc = vv[0:64, 2:3]
            nc.tensor.matmul(out=fc, lhsT=idstk, rhs=c_cc, skip_group_check=True)
            c_c = work.tile([64, 1], f32r)
            nc.scalar.activation(out=c_c, in_=fc, func=Act.Copy)

            # ---- output projection partial: y_row = c^T @ owT  [1, FIN]
            c_pr = work.tile([128, 1], f32r)
            nc.vector.tensor_copy(out=c_pr, in_=c_p)
            py0 = pmm.tile([1, 512], f32, tag="mm")
            py1 = pmm.tile([1, 256], f32, tag="mm")
            for py, (lo, hi) in ((py0, (0, 512)), (py1, (512, FIN))):
                nc.tensor.matmul(
                    out=py, lhsT=c_pr, rhs=owa[:, lo:hi], start=True, stop=False
                )
                nc.tensor.matmul(
                    out=py, lhsT=c_c, rhs=owc[:, lo:hi], start=False, stop=True
                )
            y_sb = work.tile([1, FIN], f32)
            nc.scalar.activation(out=y_sb[:, 0:512], in_=py0, func=Act.Copy)
            nc.scalar.activation(out=y_sb[:, 512:FIN], in_=py1, func=Act.Copy)
            nc.sync.dma_start(out=d_y[:, :], in_=y_sb)

    nc.finalize()
    return nc


def _make_in_maps(x, embed_w, embed_b, nnmf_w, out_w):
    def to_fp8(a):
        import ml_dtypes
        return np.ascontiguousarray(a).astype(ml_dtypes.float8_e4m3fn)

    idstk = np.zeros((128, 64), np.float32)
    for k in range(128):
        idstk[k, k % 64] = 1.0
    ones2 = np.zeros((128, 128), np.float32)
    ones2[0:64, 0:64] = 1.0
    ones2[64:128, 64:128] = 1.0

    in_maps = []
    for c in range(NCORES):
        b = c // 4
        hg = c % 4
        esl = slice(EPC * hg, EPC * (hg + 1))
        # xT packed [128, KT, S]: (p, k, t) = x[b, t, 128k+p]
        xT = np.ascontiguousarray(
            x[b].T.reshape(KT, 128, S).transpose(1, 0, 2)
        )
        ewT = np.ascontiguousarray(
            embed_w[esl, :].T.reshape(KT, 128, EPC).transpose(1, 0, 2)
        )
        ebm = embed_b[esl] - MIN_POS
        wpk = np.zeros((128, WPK_COLS), np.float32)
        wpk[:, 0] = ebm[0:128]
        wpk[0:64, 1] = ebm[128:192]
        wpk[64:128, 1] = ebm[128:192]
        wpk[0:64, 2:66] = nnmf_w
        wpk[64:128, 2:66] = nnmf_w
        wpk[:, 66:130] = idstk
        wpk[:, 130:258] = ones2
        owT = out_w[:, esl].T  # [192, FIN]
        in_maps.append({
            "xT": to_fp8(xT),
            "ewT": to_fp8(ewT * 64.0),
            "wpk": wpk,
            "owa": np.ascontiguousarray(owT[0:128, :]),
            "owc": np.ascontiguousarray(owT[128:192, :]),
        })
    return in_maps


def _ensure_ntff_hook():
    """The agent image's antenv lacks axon_hooks; synthesize it so
    run_bass_kernel_spmd(trace=True) can reach the ctypes NTFF hook."""
    import sys as _sys
    import types

    if "antenv.axon_hooks" in _sys.modules:
        return
    mod = types.ModuleType("antenv.axon_hooks")
    holder = [None]
    mod.set_axon_ntff_profile_hook = lambda h: holder.__setitem__(0, h)
    mod.get_axon_ntff_profile_hook = lambda: holder[0]
    _sys.modules["antenv.axon_hooks"] = mod
    try:
        import antenv

        antenv.axon_hooks = mod
    except ImportError:
        pass
    from trn_agent_boot.trn_boot import _ntff_profile_via_ctypes

    mod.set_axon_ntff_profile_hook(
        _ntff_profile_via_ctypes("/opt/axon/libaxon_pjrt.so")
    )


def _run(inputs, trace=False):
    from concourse import bass_utils

    if trace:
        _ensure_ntff_hook()
    if "nc" not in _CACHE:
        _CACHE["nc"] = _build_nc()
    nc = _CACHE["nc"]
    in_maps = _make_in_maps(
        inputs["x"].astype(np.float32),
        inputs["embed_w"].astype(np.float32),
        inputs["embed_b"].astype(np.float32),
        inputs["nnmf_w"].astype(np.float32),
        inputs["out_w"].astype(np.float32),
    )
    res = bass_utils.run_bass_kernel_spmd(
        nc, in_maps, core_ids=list(range(NCORES)), trace=trace
    )
    out_b = inputs["out_b"].astype(np.float32)
    y = np.zeros((B, S, FIN), np.float32)
    for bi in range(B):
        acc = np.zeros((FIN,), np.float64)
        for c in range(4 * bi, 4 * bi + 4):
            arr = np.asarray(res.results[c]["y"])  # [1, FIN]
            acc += arr.reshape(FIN)
        y[bi, :, :] = (acc + out_b).astype(np.float32)[None, :]
    return y, res


def kernel(**inputs):
    y, _ = _run(inputs, trace=False)
    return y


# revision 23
# speedup vs baseline: 1.0191x; 1.0118x over previous
"""AlphaMixerAttentionHeads TRN2 kernel (pipelined, bf16 matmul operands).

Algebraic structure (verified against the reference, inherited from the
baseline kernel):
 - alpha is i-independent (init ones, update preserves it), so it collapses
   to a per-(b,h) vector u over o; the output is constant across sequence
   positions and equals the m_3 = sum_o H3[:,o] u_3[o] channel vector.
 - W rows are L1-normalized so rec row sums equal H row sums; all per-token
   scales cancel through the NNMF recurrence, which runs on raw clipped xe:
   H_{k+1} = H_k * ((xe / (H_k @ W)) @ W^T), with H_1 = (xe * rec1r) @ W^T
   (rec1r folded into W^T's rows).
 - u_0 = 1/rowsum(H_3); hri = (rec3 * xe) / (sx * s2) feeds the
   per-iteration g = vblk^T hri matmuls; the xe clip keeps only the max
   with 0 (Relu with bias eb-MIN_POS), a ~1e-6 absolute shift that is far
   below the bf16 rounding floor.

Sharding: 8 cores; core c handles batch c//4 and heads 3*(c%4)..3*(c%4)+2
(192 embedding channels). Host sums 4 partial output projections per batch,
adds out_b, broadcasts over the sequence axis.

On-core layout is channel-major [feature, token] in one [128, 1536] tile
set: cols 0..1023 heads A,B (partitions 0-63 = A, 64-127 = B); cols
1024..1535 head C split-token (partitions 0-63 = tokens 0-511, 64-127 =
tokens 512-1023), produced directly by tile_position matmuls in the embed.
All work is chunked in 512-column pieces and software-pipelined across
PE / DVE / ACT / GpSimd. All matmul operands are bf16 (weights and moving
data; ~2.5x cheaper LDWEIGHTS+stream than f32r); PSUM accumulation and the
reciprocal/accumulate chain stay fp32, as does the output projection.
"""

import sys

sys.path.insert(0, "/opt/trn_rl_repo")

import numpy as np

B, S, FIN, E, H = 2, 1024, 768, 768, 12
DH = 64
HPC = 3          # heads per core
EPC = HPC * DH   # embed channels per core (192)
NCORES = 8
MIN_POS = 1e-6
NT = 1536        # merged token columns: 1024 pair + 512 C-split
KT = FIN // 128  # 6 contraction tiles for the embed matmul
CH = 512         # pipeline chunk columns
# wpk packed columns: ebm_p | ebm_c2 | wpair(64) | idstk(64) | ones2(128)
WPK_COLS = 1 + 1 + 64 + 64 + 128

_CACHE = {}


def _build_nc():
    import concourse.bacc as bacc
    import concourse.mybir as mybir
    from concourse.tile import TileContext

    f32 = mybir.dt.float32
    f32r = mybir.dt.float32r
    bf16 = mybir.dt.bfloat16
    Alu = mybir.AluOpType
    Act = mybir.ActivationFunctionType
    AX = mybir.AxisListType

    nc = bacc.Bacc()

    fp8 = mybir.dt.float8e4
    d_xT = nc.declare_dram_parameter("xT", [128, KT, S], fp8, isOutput=False)
    d_ewT = nc.declare_dram_parameter("ewT", [128, KT, EPC], fp8, isOutput=False)
    d_wpk = nc.declare_dram_parameter("wpk", [128, WPK_COLS], f32, isOutput=False)
    d_owa = nc.declare_dram_parameter("owa", [128, FIN], f32, isOutput=False)
    d_owc = nc.declare_dram_parameter("owc", [64, FIN], f32, isOutput=False)
    d_y = nc.declare_dram_parameter("y", [1, FIN], f32, isOutput=True)

    CHUNKS = ((0, 512), (512, 1024), (1024, 1536))

    with TileContext(nc) as tc:
        with (
            tc.tile_pool(name="const", bufs=1) as const,
            tc.tile_pool(name="xch", bufs=3) as xch,
            tc.tile_pool(name="work", bufs=1) as work,
            tc.tile_pool(name="hbuf", bufs=2) as hbuf,
            tc.tile_pool(name="tbuf", bufs=2) as tbuf,
            tc.tile_pool(name="pmm", bufs=3, space="PSUM") as pmm,
            tc.tile_pool(name="pt", bufs=2, space="PSUM") as pt,
        ):
            # ---- DMA triggers: xT on the sync queue (3), weights on the
            # scalar queue (2 now, owT later) so trigger issue is parallel.
            xts = []
            for i in range(3):
                xt = xch.tile([128, 2, S], fp8, tag="xch")
                nc.sync.dma_start(out=xt, in_=d_xT[:, 2 * i:2 * i + 2, :])
                xts.append(xt)
            ewT_sb = const.tile([128, KT, EPC], fp8)
            nc.scalar.dma_start(out=ewT_sb, in_=d_ewT[:, :, :])
            wpk = const.tile([128, WPK_COLS], f32)
            nc.scalar.dma_start(out=wpk, in_=d_wpk[:, :])

            ebm_p = wpk[:, 0:1]
            ebm_c2 = wpk[:, 1:2]
            wpair = wpk[:, 2:66]
            idstk = wpk[:, 66:130]
            ones2f = wpk[:, 130:258]

            # ---- W prep: fp32 masters on DVE, bf16 matmul copies via ACT
            W2f = const.tile([128, 128], f32)
            W2Tf = const.tile([128, 128], f32)
            Wstk2 = const.tile([128, 128], f32)
            W2b = const.tile([128, 128], bf16)
            W2Tb = const.tile([128, 128], bf16)
            W2Tpb = const.tile([128, 128], bf16)
            ones2b = const.tile([128, 128], bf16)
            vblk = const.tile([128, 128], bf16)
            vblkC = const.tile([128, 128], bf16)
            nc.scalar.activation(out=W2f, in_=wpk[:, 0:128], func=Act.Copy, scale=0.0)
            nc.scalar.activation(out=W2Tf, in_=wpk[:, 0:128], func=Act.Copy, scale=0.0)
            nc.scalar.activation(out=vblk, in_=wpk[:, 0:128], func=Act.Copy, scale=0.0)
            nc.scalar.activation(out=vblkC, in_=wpk[:, 0:128], func=Act.Copy, scale=0.0)
            nc.scalar.activation(out=ones2b, in_=ones2f, func=Act.Copy)

            wsum = work.tile([128, 1], f32)
            nc.vector.reduce_sum(out=wsum, in_=wpair, axis=AX.X)
            wrec = work.tile([128, 1], f32)
            nc.vector.reciprocal_approx_fast(out=wrec, in_=wsum)
            nc.vector.tensor_scalar(
                out=W2f[0:64, 0:64], in0=wpair[0:64, :], scalar1=wrec[0:64, :],
                scalar2=None, op0=Alu.mult,
            )
            nc.vector.tensor_scalar(
                out=W2f[64:128, 64:128], in0=wpair[64:128, :],
                scalar1=wrec[64:128, :], scalar2=None, op0=Alu.mult,
            )
            nc.vector.tensor_scalar(
                out=Wstk2[:, 0:64], in0=wpair, scalar1=wrec,
                scalar2=None, op0=Alu.mult,
            )
            nc.vector.tensor_scalar(
                out=Wstk2[:, 64:128], in0=wpair, scalar1=wrec,
                scalar2=None, op0=Alu.mult,
            )
            nc.scalar.activation(out=W2b, in_=W2f, func=Act.Copy)
            Wstk2b = const.tile([128, 128], bf16)
            nc.scalar.activation(out=Wstk2b, in_=Wstk2, func=Act.Copy)
            idstkb = const.tile([128, 64], bf16)
            nc.scalar.activation(out=idstkb, in_=idstk, func=Act.Copy)

            # ---- W2T: one PE transpose for the top block; the bottom
            # diag block is identical, replicated via a gpsimd SBUF copy.
            ps_t = pt.tile([128, 64], f32, tag="tr", bufs=1)
            nc.tensor.transpose(
                out=ps_t[0:64, :], in_=W2f[0:64, 0:64], identity=idstk[0:64, :]
            )
            nc.scalar.activation(out=W2Tf[0:64, 0:64], in_=ps_t[0:64, :], func=Act.Copy)
            nc.tensor.matmul(
                out=ps_t[64:128, :], lhsT=idstk[0:64, :],
                rhs=W2Tf[0:64, 0:64], skip_group_check=True,
            )
            nc.scalar.activation(
                out=W2Tf[64:128, 64:128], in_=ps_t[64:128, :], func=Act.Copy
            )
            nc.scalar.activation(out=W2Tb, in_=W2Tf, func=Act.Copy)

            # rec1r = 64/rowsum(W2T); W2Tp = W2T * rec1r (iter-1 fold)
            rec1s = work.tile([128, 1], f32)
            nc.vector.reduce_sum(out=rec1s, in_=W2Tf, axis=AX.X)
            rec1sc = work.tile([128, 1], f32)
            nc.vector.tensor_scalar(
                out=rec1sc, in0=rec1s, scalar1=1.0 / 64.0, scalar2=None,
                op0=Alu.mult,
            )
            rec1r = work.tile([128, 1], f32)
            nc.vector.reciprocal_approx_fast(out=rec1r, in_=rec1sc)
            nc.vector.tensor_scalar(
                out=W2Tpb, in0=W2Tf, scalar1=rec1r, scalar2=None,
                op0=Alu.mult,
            )

            # ---- embed matmuls: ep = pair heads [128, 1024];
            # psC = head C split-token [128, 512] built in place via
            # partition-offset (tile_position) matmuls.
            ep = pmm.tile([128, 1024], f32, tag="ep", bufs=1)
            psC = pmm.tile([128, CH], f32, tag="pc", bufs=1)
            DR = mybir.MatmulPerfMode.DoubleRow
            for g in range(3):
                xt = xts[g]
                lhsP2 = ewT_sb[:, 2 * g:2 * g + 2, 0:128]
                st = dict(start=(g == 0), stop=(g == 2), perf_mode=DR)
                nc.tensor.matmul(
                    out=ep[:, 0:512], lhsT=lhsP2, rhs=xt[:, :, 0:512], **st
                )
                nc.tensor.matmul(
                    out=ep[:, 512:1024], lhsT=lhsP2, rhs=xt[:, :, 512:1024], **st
                )
            for k in range(KT):
                xt = xts[k // 2][:, k % 2, :]
                lhsC = ewT_sb[:, k, 128:192]
                st2 = dict(start=(k == 0), stop=(k == KT - 1))
                nc.tensor.matmul(
                    out=psC[0:64, :], lhsT=lhsC, rhs=xt[:, 0:512],
                    skip_group_check=True, **st2,
                )
                nc.tensor.matmul(
                    out=psC[64:128, :], lhsT=lhsC, rhs=xt[:, 512:1024],
                    skip_group_check=True, **st2,
                )

            # ---- xe = relu(embed + eb - MIN_POS) on ACT (bias pre-folded)
            xe = work.tile([128, NT], bf16)
            nc.scalar.activation(
                out=xe[:, 0:512], in_=ep[:, 0:512], func=Act.Relu,
                bias=ebm_p, scale=1.0 / 64.0,
            )
            nc.scalar.activation(
                out=xe[:, 512:1024], in_=ep[:, 512:1024], func=Act.Relu,
                bias=ebm_p, scale=1.0 / 64.0,
            )
            nc.scalar.activation(
                out=xe[:, 1024:1536], in_=psC, func=Act.Relu,
                bias=ebm_c2, scale=1.0 / 64.0,
            )

            # ---- NNMF iter 1: H1 = xe @ (W^T * rec1r), plus sx row sums
            z1s = []
            for lo, hi in CHUNKS:
                z = pmm.tile([128, CH], f32, tag="mm")
                nc.tensor.matmul(out=z, lhsT=W2Tpb, rhs=xe[:, lo:hi])
                z1s.append(z)
            sxs_ps = []
            for lo, hi in CHUNKS:
                sx = pmm.tile([128, CH], f32, tag="mm")
                nc.tensor.matmul(out=sx, lhsT=ones2b, rhs=xe[:, lo:hi])
                sxs_ps.append(sx)
            Hc = hbuf.tile([128, NT], bf16, tag="h")
            for ci, (lo, hi) in enumerate(CHUNKS):
                nc.scalar.activation(out=Hc[:, lo:hi], in_=z1s[ci], func=Act.Copy)
            sxs = work.tile([128, NT], f32)
            for ci, (lo, hi) in enumerate(CHUNKS):
                nc.scalar.activation(out=sxs[:, lo:hi], in_=sxs_ps[ci], func=Act.Copy)

            # ---- NNMF iters 2-3 (chunk-pipelined); iter 3 also builds hri
            owa = const.tile([128, FIN], f32r)
            owc = const.tile([64, FIN], f32r)
            rec3s = work.tile([128, NT], bf16)
            sxs2 = work.tile([128, NT], f32)
            hri_raw = work.tile([128, NT], bf16)
            hri = work.tile([128, NT], bf16)
            for it in range(1, 3):
                last = it == 2
                recs = []
                for lo, hi in CHUNKS:
                    rec = pmm.tile([128, CH], f32, tag="mm")
                    nc.tensor.matmul(out=rec, lhsT=W2b, rhs=Hc[:, lo:hi])
                    recs.append(rec)
                q = work.tile([128, NT], bf16, tag="q")
                rr = work.tile([128, NT], f32, tag="rr")
                for ci, (lo, hi) in enumerate(CHUNKS):
                    nc.vector.reciprocal_approx_fast(out=rr[:, lo:hi], in_=recs[ci])
                    on_pool = ci == 2 or (ci == 1 and not last)
                    eng = nc.gpsimd if on_pool else nc.vector
                    eng.tensor_tensor(
                        out=q[:, lo:hi], in0=xe[:, lo:hi], in1=rr[:, lo:hi],
                        op=Alu.mult,
                    )
                if last:
                    for ci, (lo, hi) in enumerate(CHUNKS):
                        nc.scalar.activation(
                            out=rec3s[:, lo:hi], in_=recs[ci], func=Act.Copy
                        )
                    nc.scalar.dma_start(out=owa, in_=d_owa[:, :].bitcast(f32r))
                    nc.scalar.dma_start(out=owc, in_=d_owc[:, :].bitcast(f32r))
                zs = []
                for lo, hi in CHUNKS:
                    z = pmm.tile([128, CH], f32, tag="mm")
                    nc.tensor.matmul(out=z, lhsT=W2Tb, rhs=q[:, lo:hi])
                    zs.append(z)
                Hn = hbuf.tile([128, NT], bf16, tag="h")
                for ci, (lo, hi) in enumerate(CHUNKS):
                    nc.vector.tensor_tensor(
                        out=Hn[:, lo:hi], in0=Hc[:, lo:hi],
                        in1=zs[ci], op=Alu.mult,
                    )
                if not last:
                    # s2 row sums of H2 (side path for hri)
                    s2_ps = []
                    for lo, hi in CHUNKS:
                        s2 = pmm.tile([128, CH], f32, tag="mm")
                        nc.tensor.matmul(out=s2, lhsT=ones2b, rhs=Hn[:, lo:hi])
                        s2_ps.append(s2)
                    for ci, (lo, hi) in enumerate(CHUNKS):
                        nc.vector.tensor_tensor(
                            out=sxs2[:, lo:hi], in0=sxs[:, lo:hi],
                            in1=s2_ps[ci], op=Alu.mult,
                        )
                else:
                    # hri = (rec3 * xe) / (sx * s2)
                    for lo, hi in CHUNKS:
                        nc.gpsimd.tensor_tensor(
                            out=hri_raw[:, lo:hi], in0=rec3s[:, lo:hi],
                            in1=xe[:, lo:hi], op=Alu.mult,
                        )
                    rho2 = work.tile([128, NT], f32)
                    for lo, hi in CHUNKS:
                        nc.vector.reciprocal_approx_fast(
                            out=rho2[:, lo:hi], in_=sxs2[:, lo:hi]
                        )
                    for ci, (lo, hi) in enumerate(CHUNKS):
                        eng = nc.gpsimd if ci == 2 else nc.vector
                        eng.tensor_tensor(
                            out=hri[:, lo:hi], in0=hri_raw[:, lo:hi],
                            in1=rho2[:, lo:hi], op=Alu.mult,
                        )
                Hc = Hn

            # ---- s3 row sums of H3, u0 = 1/s3
            s3_ps = []
            for lo, hi in CHUNKS:
                s3 = pmm.tile([128, CH], f32, tag="mm")
                nc.tensor.matmul(out=s3, lhsT=ones2b, rhs=Hc[:, lo:hi])
                s3_ps.append(s3)
            u0 = work.tile([128, NT], f32)
            for ci, (lo, hi) in enumerate(CHUNKS):
                nc.vector.reciprocal_approx_fast(out=u0[:, lo:hi], in_=s3_ps[ci])

            # ---- alpha fixed point: 4 accumulation passes, 3 v/g rounds
            vv = pt.tile([128, 4], f32, tag="v", bufs=1)
            c_p = work.tile([128, 1], f32)
            c_cc = work.tile([128, 1], f32)
            t_prev = None
            g_ps = None
            for it in range(4):
                lastit = it == 3
                t = tbuf.tile([128, NT], f32, tag="t")
                in0 = Hc if it == 0 else t_prev
                macc = []
                for ci, (lo, hi) in enumerate(CHUNKS):
                    in1 = u0[:, lo:hi] if it == 0 else g_ps[ci]
                    m = work.tile([128, 1], f32, tag=f"m{it}{ci}")
                    nc.vector.scalar_tensor_tensor(
                        out=t[:, lo:hi], in0=in0[:, lo:hi], scalar=1.0,
                        in1=in1, op0=Alu.mult, op1=Alu.mult, accum_out=m,
                    )
                    macc.append(m)
                t_prev = t
                m_cc = macc[2]
                if lastit:
                    nc.vector.tensor_tensor(
                        out=c_p, in0=macc[0], in1=macc[1], op=Alu.add
                    )
                    nc.vector.tensor_copy(out=c_cc, in_=m_cc)
                    break
                m_pb = work.tile([128, 1], bf16, tag=f"mpb{it}")
                nc.vector.tensor_tensor(
                    out=m_pb, in0=macc[0], in1=macc[1], op=Alu.add
                )
                m_cb = work.tile([128, 1], bf16, tag=f"mcb{it}")
                nc.vector.tensor_copy(out=m_cb, in_=m_cc)
                vps = vv[:, 0:1]
                nc.tensor.matmul(out=vps, lhsT=W2b, rhs=m_pb, skip_group_check=True)
                vcs = vv[:, 1:2]
                nc.tensor.matmul(out=vcs, lhsT=Wstk2b, rhs=m_cb, skip_group_check=True)
                v_p = work.tile([128, 1], f32, tag="v_p")
                v_c = work.tile([128, 1], f32, tag="v_c")
                nc.vector.reciprocal_approx_fast(out=v_p, in_=vps)
                nc.vector.reciprocal_approx_fast(out=v_c, in_=vcs)
                nc.scalar.activation(
                    out=vblk, in_=ones2f, func=Act.Copy, scale=v_p
                )
                nc.scalar.activation(
                    out=vblkC, in_=ones2f, func=Act.Copy, scale=v_c
                )
                g_ps = []
                for ci, (lo, hi) in enumerate(CHUNKS):
                    g = pmm.tile([128, CH], f32, tag="mm")
                    nc.tensor.matmul(
                        out=g, lhsT=(vblkC if ci == 2 else vblk),
                        rhs=hri[:, lo:hi],
                    )
                    g_ps.append(g)

            # fold the C accumulator's split halves: c_c[f] = acc[f]+acc[64+f]
            c_ccb = work.tile([128, 1], bf16)
            nc.vector.tensor_copy(out=c_ccb, in_=c_cc)
            fc = vv[0:64, 2:3]
            nc.tensor.matmul(out=fc, lhsT=idstkb, rhs=c_ccb, skip_group_check=True)
            c_c = work.tile([64, 1], f32r)
            nc.scalar.activation(out=c_c, in_=fc, func=Act.Copy)

            # ---- output projection partial: y_row = c^T @ owT  [1, FIN]
            c_pr = work.tile([128, 1], f32r)
            nc.vector.tensor_copy(out=c_pr, in_=c_p)
            py0 = pmm.tile([1, 512], f32, tag="mm")
            py1 = pmm.tile([1, 256], f32, tag="mm")
            for py, (lo, hi) in ((py0, (0, 512)), (py1, (512, FIN))):
                nc.tensor.matmul(
                    out=py, lhsT=c_pr, rhs=owa[:, lo:hi], start=True, stop=False
                )
                nc.tensor.matmul(
                    out=py, lhsT=c_c, rhs=owc[:, lo:hi], start=False, stop=True
                )
            y_sb = work.tile([1, FIN], f32)
            nc.scalar.activation(out=y_sb[:, 0:512], in_=py0, func=Act.Copy)
            nc.scalar.activation(out=y_sb[:, 512:FIN], in_=py1, func=Act.Copy)
            nc.sync.dma_start(out=d_y[:, :], in_=y_sb)

    nc.finalize()
    return nc


def _make_in_maps(x, embed_w, embed_b, nnmf_w, out_w):
    def to_fp8(a):
        import ml_dtypes
        return np.ascontiguousarray(a).astype(ml_dtypes.float8_e4m3fn)

    idstk = np.zeros((128, 64), np.float32)
    for k in range(128):
        idstk[k, k % 64] = 1.0
    ones2 = np.zeros((128, 128), np.float32)
    ones2[0:64, 0:64] = 1.0
    ones2[64:128, 64:128] = 1.0

    in_maps = []
    for c in range(NCORES):
        b = c // 4
        hg = c % 4
        esl = slice(EPC * hg, EPC * (hg + 1))
        # xT packed [128, KT, S]: (p, k, t) = x[b, t, 128k+p]
        xT = np.ascontiguousarray(
            x[b].T.reshape(KT, 128, S).transpose(1, 0, 2)
        )
        ewT = np.ascontiguousarray(
            embed_w[esl, :].T.reshape(KT, 128, EPC).transpose(1, 0, 2)
        )
        ebm = embed_b[esl] - MIN_POS
        wpk = np.zeros((128, WPK_COLS), np.float32)
        wpk[:, 0] = ebm[0:128]
        wpk[0:64, 1] = ebm[128:192]
        wpk[64:128, 1] = ebm[128:192]
        wpk[0:64, 2:66] = nnmf_w
        wpk[64:128, 2:66] = nnmf_w
        wpk[:, 66:130] = idstk
        wpk[:, 130:258] = ones2
        owT = out_w[:, esl].T  # [192, FIN]
        in_maps.append({
            "xT": to_fp8(xT),
            "ewT": to_fp8(ewT * 64.0),
            "wpk": wpk,
            "owa": np.ascontiguousarray(owT[0:128, :]),
            "owc": np.ascontiguousarray(owT[128:192, :]),
        })
    return in_maps


def _ensure_ntff_hook():
    """The agent image's antenv lacks axon_hooks; synthesize it so
    run_bass_kernel_spmd(trace=True) can reach the ctypes NTFF hook."""
    import sys as _sys
    import types

    if "antenv.axon_hooks" in _sys.modules:
        return
    mod = types.ModuleType("antenv.axon_hooks")
    holder = [None]
    mod.set_axon_ntff_profile_hook = lambda h: holder.__setitem__(0, h)
    mod.get_axon_ntff_profile_hook = lambda: holder[0]
    _sys.modules["antenv.axon_hooks"] = mod
    try:
        import antenv

        antenv.axon_hooks = mod
    except ImportError:
        pass
    from trn_agent_boot.trn_boot import _ntff_profile_via_ctypes

    mod.set_axon_ntff_profile_hook(
        _ntff_profile_via_ctypes("/opt/axon/libaxon_pjrt.so")
    )


def _run(inputs, trace=False):
    from concourse import bass_utils

    if trace:
        _ensure_ntff_hook()
    if "nc" not in _CACHE:
        _CACHE["nc"] = _build_nc()
    nc = _CACHE["nc"]
    in_maps = _make_in_maps(
        inputs["x"].astype(np.float32),
        inputs["embed_w"].astype(np.float32),
        inputs["embed_b"].astype(np.float32),
        inputs["nnmf_w"].astype(np.float32),
        inputs["out_w"].astype(np.float32),
    )
    res = bass_utils.run_bass_kernel_spmd(
        nc, in_maps, core_ids=list(range(NCORES)), trace=trace
    )
    out_b = inputs["out_b"].astype(np.float32)
    y = np.zeros((B, S, FIN), np.float32)
    for bi in range(B):
        acc = np.zeros((FIN,), np.float64)
        for c in range(4 * bi, 4 * bi + 4):
            arr = np.asarray(res.results[c]["y"])  # [1, FIN]
            acc += arr.reshape(FIN)
        y[bi, :, :] = (acc + out_b).astype(np.float32)[None, :]
    return y, res


def kernel(**inputs):
    y, _ = _run(inputs, trace=False)
    return y
